# revision 6
# baseline (speedup 1.0000x reference)
"""Distributed causal RoPE attention for Trainium2 (8 NeuronCores), v2.

Mesh: 2 (batch) x 4 (head-group tensor-parallel).
Core c = b*4 + g handles batch b, heads [4g, 4g+4).

v2 vs v1 (702 us): bf16 matmul/operand dtypes everywhere (fp32 PSUM
accumulation), transposed-score attention that eliminates all 544
per-tile attention transposes and their PSUM->SBUF casts, quad-width
(512-col) attention tiles so LDWEIGHTS hides under matmuls,
unnormalized softmax with the denominator produced by a ones-matmul
broadcast + reciprocal folded into the PSUM drain, work spread across
Scalar/Pool engines, and a bf16 ReduceScatter (half the link bytes).

Per core:
  - QKV projections in bf16 (x^T resident in SBUF), RoPE on DVE in bf16
    at 2x rate (head dims pre-permuted to even|odd halves via host-side
    column permutation of Wq/Wk), PE transposes to Q^T/K^T [d=128, S]
  - attention per head per quad (512 q cols): S^T tiles [k=128, q=512]
    = K_tile^T.T @ Q^T directly (no transposes), exp on Scalar with
    fused scale, within-quad causal masking by 0/1 bf16 mask multiply
    AFTER exp, denominator = ones-matmul over a DVE/Pool-accumulated
    tile sum, PV matmul produces O^T unnormalized, normalization fused
    into the PSUM->SBUF drain (mul by broadcast reciprocal)
  - output projection accumulating the 4 heads in PSUM (bf16 weights)
  - chunked bf16 ReduceScatter(add) over the 4-core group; quad order
    (0,1,3,2) so the two small tail chunks fire early
Host reassembles the full [2, 2048, 2048] fp32 output from the shards.
"""

import sys

sys.path.insert(0, "/opt/trn_rl_repo")

import numpy as np
import ml_dtypes

import concourse.bass as bass
import concourse.mybir as mybir
import concourse.tile as tile
from concourse.bass_utils import run_bass_kernel_spmd
from concourse.tile import add_dep_helper

FP = mybir.dt.float32
FR = mybir.dt.float32r
BF = mybir.dt.bfloat16
D = 2048  # d_model
S = 2048  # sequence length
B = 2  # batch
NH = 16  # heads
DKV = 128  # head dim
THETA = 10000.0
TP = 4  # head-parallel groups
HPC = NH // TP  # heads per core = 4
HD = HPC * DKV  # head dims per core = 512
NQT = S // 128  # 16 s-tiles
NDC = D // 128  # 16 contraction chunks
NQD = NQT // 4  # 4 quads of 4 q-tiles
SCALE = 1.0 / float(np.sqrt(DKV))
N_CORES = 8

BF_NP = ml_dtypes.bfloat16


def _legalize_waits(nc):
    """This walrus build only accepts one embedded sync-wait per TPB
    instruction ("Too many sync wait commands").  Split excess waits of
    compute-engine instructions into preceding engine-local NoOps, each
    carrying a single wait.  DMA (queue-embedded) waits are left alone.
    """
    n_split = 0
    for f in nc.m.functions:
        for bb in f.blocks:
            out = []
            for ins in bb.instructions:
                si = ins.sync_info
                if (
                    si is not None
                    and len(si.on_wait) > 1
                    and ins.engine != mybir.EngineType.Unassigned
                ):
                    waits = {}
                    for w in si.on_wait:
                        key = (w.sync_type, w.id, w.wait_mode)
                        if key not in waits or (
                            w.wait_value is not None
                            and waits[key].wait_value is not None
                            and w.wait_value > waits[key].wait_value
                        ):
                            waits[key] = w
                    waits = list(waits.values())
                    for w in waits[:-1]:
                        nop = mybir.InstNoOp(name=f"{ins.name}-waitsplit-{n_split}")
                        n_split += 1
                        nop.engine = ins.engine
                        nop.sync_info = mybir.SyncInfo(on_wait=[w], on_update=[])
                        out.append(nop)
                    ins.sync_info = mybir.SyncInfo(
                        on_wait=[waits[-1]], on_update=si.on_update
                    )
                out.append(ins)
            bb.instructions = out
    return n_split


RS_CHUNKS = [4, 4, 4, 4]  # one chunk per quad
QUAD_ORDER = (3, 2, 1, 0)  # biggest quad first: the trailing quad is the cheapest


def build_nc():
    nc = bass.Bass()

    xT = nc.declare_dram_parameter("xT", [NQT, NDC, 128, 128], BF, isOutput=False)
    wq = nc.declare_dram_parameter("wq", [D, HD], BF, isOutput=False)
    wk = nc.declare_dram_parameter("wk", [D, HD], BF, isOutput=False)
    wv = nc.declare_dram_parameter("wv", [D, HD], BF, isOutput=False)
    wo = nc.declare_dram_parameter("wo", [HD, D], BF, isOutput=False)
    cosp = nc.declare_dram_parameter("cosp", [S, DKV // 2], BF, isOutput=False)
    sinp = nc.declare_dram_parameter("sinp", [S, DKV // 2], BF, isOutput=False)
    mq = nc.declare_dram_parameter("mq", [4, 128, 512], BF, isOutput=False)
    identp = nc.declare_dram_parameter("identp", [128, 128], BF, isOutput=False)
    onesp = nc.declare_dram_parameter("onesp", [128, 128], FR, isOutput=False)
    out = nc.declare_dram_parameter("out", [S // TP, D], BF, isOutput=True)

    rs_bound = []
    acc = 0
    for n in RS_CHUNKS:
        acc += n
        rs_bound.append(acc - 1)  # last q-tile index of each chunk

    with tile.TileContext(nc) as tc:
        with (
            tc.tile_pool(name="dram", bufs=1, space="DRAM") as dram,
            tc.tile_pool(name="const", bufs=1) as constp,
            tc.tile_pool(name="resident", bufs=1) as resp,
        ):
            partials = [
                dram.tile([n * 128, D], BF, name=f"partial{c}", tag=f"partial{c}")
                for c, n in enumerate(RS_CHUNKS)
            ]
            rs_outs = [
                dram.tile([n * 32, D], BF, name=f"rs_out{c}", tag=f"rs_out{c}")
                for c, n in enumerate(RS_CHUNKS)
            ]

            ident = constp.tile([128, 128], BF, tag="ident")
            ones_fr = constp.tile([128, 128], FR, tag="ones_fr")
            mask_sb = constp.tile([128, 4 * 512], BF, tag="mask")
            cos_sb = constp.tile([128, NQT * 64], BF, tag="cos")
            sin_sb = constp.tile([128, NQT * 64], BF, tag="sin")

            # Q^T/K^T: [128 (head dim, even|odd basis), HPC*S]; block (h, st)
            # at free offset h*S + st*128.  V: [128 (= k within chunk), HPC*S]
            # block (h, kc) holds V[k-chunk kc, dims of head h] (natural basis).
            QT = resp.tile([128, HPC * S], BF, tag="QT")
            KT = resp.tile([128, HPC * S], BF, tag="KT")
            V = resp.tile([128, HPC * S], BF, tag="V")
            wo_sb = resp.tile([128, HPC * D], BF, tag="wo")

            # ---------------- QKV projection phase ----------------
            with (
                tc.tile_pool(name="xpool", bufs=1) as xpool,
                tc.tile_pool(name="wpool", bufs=1) as wpool,
                tc.tile_pool(name="ropep", bufs=3) as ropep,
                tc.tile_pool(name="qps", bufs=3, space="PSUM") as qps,
                tc.tile_pool(name="trps", bufs=4, space="PSUM") as trps,
            ):
                # per-chunk tiles so dependencies are slice-granular and
                # DMA issue order matches PE consumption order
                wq_t = [wpool.tile([128, HD], BF, name=f"wq{dc}", tag=f"wq{dc}") for dc in range(NDC)]
                wk_t = [wpool.tile([128, HD], BF, name=f"wk{dc}", tag=f"wk{dc}") for dc in range(NDC)]
                wv_t = [wpool.tile([128, HD], BF, name=f"wv{dc}", tag=f"wv{dc}") for dc in range(NDC)]
                x_t = [
                    xpool.tile([128, NDC * 128], BF, name=f"x{st}", tag=f"x{st}")
                    for st in range(NQT)
                ]
                def w_dma(w_t, w, lo, hi):
                    for d0 in range(lo, hi, 4):
                        for dc in range(d0, d0 + 4):
                            nc.sync.dma_start(
                                w_t[dc][:], w[dc * 128 : (dc + 1) * 128, :]
                            )

                def x_dma(st):
                    nc.sync.dma_start(
                        x_t[st][:].rearrange("p (c s) -> p c s", s=128),
                        xT[st].rearrange("c p s -> p c s"),
                    )

                w_dma(wq_t, wq, 0, NDC)
                x_dma(0)
                x_dma(1)
                nc.sync.dma_start(
                    cos_sb[:].rearrange("p (t f) -> p t f", f=64),
                    cosp[:, :].rearrange("(t p) f -> p t f", p=128),
                )
                nc.sync.dma_start(
                    sin_sb[:].rearrange("p (t f) -> p t f", f=64),
                    sinp[:, :].rearrange("(t p) f -> p t f", p=128),
                )
                nc.sync.dma_start(ident[:], identp[:, :])
                for st in range(2, NQT):
                    x_dma(st)
                w_dma(wk_t, wk, 0, NDC)
                w_dma(wv_t, wv, 0, NDC)
                nc.sync.dma_start(
                    wo_sb[:].rearrange("p (h d) -> p h d", h=HPC),
                    wo[:, :].rearrange("(h p) d -> p h d", p=128),
                )
                nc.sync.dma_start(
                    mask_sb[:].rearrange("p (j q) -> p j q", j=4), mq[:, :, :].rearrange("j p q -> p j q")
                )
                nc.sync.dma_start(ones_fr[:], onesp[:, :])

                def rope_block(ps, st):
                    qtmp = ropep.tile([128, HD], BF, tag="qtmp")
                    nc.scalar.activation(
                        qtmp[:], ps[:], mybir.ActivationFunctionType.Copy
                    )
                    rot = ropep.tile([128, HD], BF, tag="rot")
                    tmp = ropep.tile([128, HD], BF, tag="tmp")
                    cc = (
                        cos_sb[:, st * 64 : (st + 1) * 64]
                        .rearrange("p (o f) -> p o f", o=1)
                        .broadcast_to((128, HPC, 64))
                    )
                    ss = (
                        sin_sb[:, st * 64 : (st + 1) * 64]
                        .rearrange("p (o f) -> p o f", o=1)
                        .broadcast_to((128, HPC, 64))
                    )
                    qv2 = qtmp[:].rearrange("p (h f) -> p h f", h=HPC)
                    rotv = rot[:].rearrange("p (h f) -> p h f", h=HPC)
                    tmpv = tmp[:].rearrange("p (h f) -> p h f", h=HPC)
                    x1 = qv2[:, :, 0:64]
                    x2 = qv2[:, :, 64:128]
                    t1 = tmpv[:, :, 0:64]
                    t2 = tmpv[:, :, 64:128]
                    nc.vector.tensor_mul(t1, x1, cc)
                    nc.vector.tensor_mul(t2, x2, ss)
                    nc.vector.tensor_sub(rotv[:, :, 0:64], t1, t2)
                    nc.vector.tensor_mul(t1, x1, ss)
                    nc.vector.tensor_mul(t2, x2, cc)
                    nc.vector.tensor_add(rotv[:, :, 64:128], t1, t2)
                    return rot

                def transpose_block(rot, st, dst):
                    for h in range(HPC):
                        pt = trps.tile([128, 128], BF, tag="tr")
                        nc.tensor.transpose(
                            pt[:], rot[:, h * 128 : (h + 1) * 128], ident[:]
                        )
                        nc.scalar.activation(
                            dst[:, h * S + st * 128 : h * S + (st + 1) * 128],
                            pt[:],
                            mybir.ActivationFunctionType.Copy,
                        )

                for w_t, dst in ((wq_t, QT), (wk_t, KT)):
                    pending = None  # (rot, st) whose transposes are deferred
                    for st in range(NQT):
                        ps = qps.tile([128, HD], FP, tag="qkv")
                        xvst = x_t[st][:].rearrange("p (c s) -> p c s", s=128)
                        for dc in range(NDC):
                            nc.tensor.matmul(
                                ps[:, :],
                                xvst[:, dc],
                                w_t[dc][:],
                                start=(dc == 0),
                                stop=(dc == NDC - 1),
                            )
                        if pending is not None:
                            transpose_block(*pending, dst)
                        pending = (rope_block(ps, st), st)
                    transpose_block(*pending, dst)
                for st in range(NQT):
                    ps = qps.tile([128, HD], FP, tag="qkv")
                    xvst = x_t[st][:].rearrange("p (c s) -> p c s", s=128)
                    for dc in range(NDC):
                        nc.tensor.matmul(
                            ps[:, :],
                            xvst[:, dc],
                            wv_t[dc][:],
                            start=(dc == 0),
                            stop=(dc == NDC - 1),
                        )
                    nc.scalar.activation(
                        V[:].rearrange("p (h t s) -> p h t s", h=HPC, t=NQT)[
                            :, :, st, :
                        ],
                        ps[:].rearrange("p (h s) -> p h s", h=HPC),
                        mybir.ActivationFunctionType.Copy,
                    )

            # ---------------- attention + output projection ----------------
            with (
                tc.tile_pool(name="atp", bufs=6) as atp,
                tc.tile_pool(name="accp", bufs=3) as accp,
                tc.tile_pool(name="rcpp", bufs=2) as rcpp,
                tc.tile_pool(name="ATp", bufs=2) as ATp,
                tc.tile_pool(name="outp", bufs=6) as outp,
                tc.tile_pool(name="scp2", bufs=3, space="PSUM") as scp2,
                tc.tile_pool(name="ops", bufs=2, space="PSUM") as ops,
            ):
                for qd in QUAD_ORDER:
                    T = 4 * qd + 4  # kt tiles for this quad
                    # per-head O^T tiles for q cols [qd*512, qd*512+512)
                    ATh = [
                        ATp.tile([128, 512], BF, name=f"AT{h}", tag=f"AT{h}")
                        for h in range(HPC)
                    ]
                    # heads processed in pairs: one [128, 1024] exp per kt
                    # keeps the Scalar engine at the PE's pace
                    for hp in (0, 2):
                        hh = (hp, hp + 1)
                        ps_o_t = [
                            ops.tile([128, 512], FP, name=f"pv{h}", tag="pv")
                            for h in hh
                        ]
                        ps_o = [t[:] for t in ps_o_t]
                        acc2_t = accp.tile([128, 1024], FR, name="acc2", tag="acc")
                        acc2 = acc2_t[:]
                        LAG = 3

                        def issue_pv(kt, at2):
                            a2v = at2[:].rearrange("p (g q) -> p g q", g=2)
                            for i, h in enumerate(hh):
                                nc.tensor.matmul(
                                    ps_o[i],
                                    V[:, h * S + kt * 128 : h * S + (kt + 1) * 128],
                                    a2v[:, i],
                                    start=(kt == 0),
                                    stop=(kt == T - 1),
                                )

                        pend = []
                        at2_even = None
                        pair2_prev = None
                        for kt in range(T):
                            sc2 = scp2.tile([128, 1024], FP, tag="sc2")
                            for i, h in enumerate(hh):
                                nc.tensor.matmul(
                                    sc2[:, i * 512 : (i + 1) * 512],
                                    KT[:, h * S + kt * 128 : h * S + (kt + 1) * 128],
                                    QT[:, h * S + qd * 512 : h * S + (qd + 1) * 512],
                                    start=True,
                                    stop=True,
                                )
                            at2 = atp.tile([128, 1024], BF, tag="at")
                            j = kt - 4 * qd
                            s2v = sc2[:].rearrange("p (g q) -> p g q", g=2)
                            a2v = at2[:].rearrange("p (g q) -> p g q", g=2)
                            if j > 0:
                                # left j*128 q-cols are fully causal-masked:
                                # skip them in the exp, zero them instead
                                nc.scalar.activation(
                                    a2v[:, :, j * 128 :],
                                    s2v[:, :, j * 128 :],
                                    mybir.ActivationFunctionType.Exp,
                                    bias=0.0,
                                    scale=SCALE,
                                )
                                nc.vector.memset(a2v[:, :, : j * 128], 0)
                            else:
                                nc.scalar.activation(
                                    at2[:],
                                    sc2[:],
                                    mybir.ActivationFunctionType.Exp,
                                    bias=0.0,
                                    scale=SCALE,
                                )
                            if j >= 0:
                                # diagonal 128-wide triangular block
                                mbc = (
                                    mask_sb[
                                        :, j * 512 + j * 128 : j * 512 + (j + 1) * 128
                                    ]
                                    .rearrange("p (o q) -> p o q", o=1)
                                    .broadcast_to((128, 2, 128))
                                )
                                dg = a2v[:, :, j * 128 : (j + 1) * 128]
                                nc.vector.tensor_mul(dg, dg, mbc)
                            # 2-level bf16 pair tree, fp32 accumulate per 4 kt
                            if kt % 2 == 0:
                                at2_even = at2
                            else:
                                pair2 = atp.tile([128, 1024], BF, tag="pair")
                                nc.vector.tensor_add(pair2[:], at2_even[:], at2[:])
                                if kt % 4 == 1:
                                    pair2_prev = pair2
                                else:
                                    quad2 = atp.tile([128, 1024], BF, tag="quad")
                                    nc.vector.tensor_add(
                                        quad2[:], pair2_prev[:], pair2[:]
                                    )
                                    if kt == 3:
                                        nc.vector.tensor_copy(acc2, quad2[:])
                                    else:
                                        nc.vector.tensor_add(acc2, acc2, quad2[:])
                            pend.append((kt, at2))
                            if len(pend) > LAG:
                                issue_pv(*pend.pop(0))
                        for item in pend:
                            issue_pv(*item)
                        # denominator broadcast [128, 1024] = ones^T @ acc2
                        dn2 = scp2.tile([128, 1024], FP, tag="sc2")
                        for i in range(2):
                            nc.tensor.matmul(
                                dn2[:, i * 512 : (i + 1) * 512],
                                ones_fr[:],
                                acc2[:, i * 512 : (i + 1) * 512],
                                start=True,
                                stop=True,
                            )
                        rcp2 = rcpp.tile([128, 1024], FP, tag="rcp")
                        nc.vector.reciprocal(rcp2[:], dn2[:])
                        # normalization fused into the PSUM drain
                        for i, h in enumerate(hh):
                            nc.vector.tensor_mul(
                                ATh[h][:],
                                ps_o[i],
                                rcp2[:, i * 512 : (i + 1) * 512],
                            )
                    # output projection for the quad's 4 q-tiles
                    for qi in range(4):
                        qt = 4 * qd + qi
                        for nt in range(D // 512):
                            ps_p = ops.tile([128, 512], FP, tag="pv")
                            for h in range(HPC):
                                nc.tensor.matmul(
                                    ps_p[:],
                                    ATh[h][:, qi * 128 : (qi + 1) * 128],
                                    wo_sb[:, h * D + nt * 512 : h * D + (nt + 1) * 512],
                                    start=(h == 0),
                                    stop=(h == HPC - 1),
                                )
                            osb = outp.tile([128, 512], BF, tag="osb")
                            nc.scalar.activation(
                                osb[:], ps_p[:], mybir.ActivationFunctionType.Copy
                            )
                            c = next(
                                i for i, bnd in enumerate(rs_bound) if qt <= bnd
                            )
                            qoff = qt - (rs_bound[c] - RS_CHUNKS[c] + 1)
                            last_partial_dma = nc.sync.dma_start(
                                partials[c][
                                    qoff * 128 : (qoff + 1) * 128,
                                    nt * 512 : (nt + 1) * 512,
                                ],
                                osb[:],
                            )
                        # fire each finished RS chunk (gpsimd so its wait
                        # never stalls the Sync engine's in-order DMA stream)
                        if qt in rs_bound:
                            c = rs_bound.index(qt)
                            nc.gpsimd.collective_compute(
                                "ReduceScatter",
                                mybir.AluOpType.add,
                                replica_groups=[[0, 1, 2, 3], [4, 5, 6, 7]],
                                ins=[partials[c].opt()],
                                outs=[rs_outs[c].opt()],
                            )
                # final out-DMAs last so no engine stream ever stalls
                # mid-kernel waiting on a collective; order matches RS
                # completion order
                for c in (3, 2, 1, 0):
                    ooff = sum(n * 32 for n in RS_CHUNKS[:c])
                    od = nc.sync.dma_start(
                        out[ooff : ooff + RS_CHUNKS[c] * 32, :], rs_outs[c][:, :]
                    )
                    add_dep_helper(od.ins, last_partial_dma.ins, False, "out-dma last")

    import os

    if not os.environ.get("BASS_NO_LEGALIZE"):
        n = _legalize_waits(nc)
        print(f"kernel: split {n} excess sync waits", file=sys.stderr)
    return nc


_NC_CACHE = None
LAST_RESULTS = None


def _ensure_ntff_hook():
    """The agent image's antenv lacks ``axon_hooks``, so the boot-time NTFF
    profile hook registration silently degrades and ``trace=True`` crashes
    on import.  Recreate the module and register the ctypes hook."""
    try:
        from antenv.axon_hooks import get_axon_ntff_profile_hook  # noqa: F401

        return
    except ImportError:
        pass
    import types

    import antenv

    mod = types.ModuleType("antenv.axon_hooks")
    _hook = [None]
    mod.set_axon_ntff_profile_hook = lambda h: _hook.__setitem__(0, h)
    mod.get_axon_ntff_profile_hook = lambda: _hook[0]
    sys.modules["antenv.axon_hooks"] = mod
    antenv.axon_hooks = mod
    if "/root/.axon_site" not in sys.path:
        sys.path.insert(0, "/root/.axon_site")
    from trn_agent_boot.trn_boot import _ntff_profile_via_ctypes

    mod.set_axon_ntff_profile_hook(
        _ntff_profile_via_ctypes("/opt/axon/libaxon_pjrt.so")
    )


def _get_nc():
    global _NC_CACHE
    if _NC_CACHE is None:
        _NC_CACHE = build_nc()
    return _NC_CACHE


def _shard_inputs(x, Wq, Wk, Wv, Wo, token_position):
    x = np.asarray(x, dtype=np.float32)
    Wq = np.asarray(Wq, dtype=np.float32)
    Wk = np.asarray(Wk, dtype=np.float32)
    Wv = np.asarray(Wv, dtype=np.float32)
    Wo = np.asarray(Wo, dtype=np.float32)
    pos = np.asarray(token_position)

    inv_freq = (1.0 / (THETA ** (np.arange(0, DKV, 2, dtype=np.float32) / DKV))).astype(
        np.float32
    )
    ang = pos.astype(np.float32)[:, None] * inv_freq[None, :]
    cos = np.cos(ang).astype(BF_NP)
    sin = np.sin(ang).astype(BF_NP)

    # within-quad causal 0/1 masks: tile j valid iff q' >= j*128 + k
    qs = np.arange(512)[None, :]
    ks = np.arange(128)[:, None]
    mq = np.stack(
        [(qs >= (j * 128 + ks)) for j in range(4)], axis=0
    ).astype(BF_NP)
    ident = np.eye(128, dtype=BF_NP)
    ones = np.ones((128, 128), dtype=np.float32)

    # per-head even|odd column permutation for RoPE half-split basis
    perm1 = np.concatenate([np.arange(0, DKV, 2), np.arange(1, DKV, 2)])
    xTb = [
        np.ascontiguousarray(
            x[b].T.reshape(NDC, 128, NQT, 128).transpose(2, 0, 1, 3)
        ).astype(BF_NP)
        for b in range(B)
    ]
    in_maps = []
    for c in range(N_CORES):
        b, g = divmod(c, TP)
        hs = slice(g * HD, (g + 1) * HD)
        permg = np.concatenate([h * DKV + perm1 for h in range(HPC)])
        in_maps.append(
            {
                "xT": xTb[b],
                "wq": np.ascontiguousarray(Wq[:, hs][:, permg]).astype(BF_NP),
                "wk": np.ascontiguousarray(Wk[:, hs][:, permg]).astype(BF_NP),
                "wv": np.ascontiguousarray(Wv[:, hs]).astype(BF_NP),
                "wo": np.ascontiguousarray(Wo[hs, :]).astype(BF_NP),
                "cosp": cos,
                "sinp": sin,
                "mq": mq,
                "identp": ident,
                "onesp": ones,
            }
        )
    return in_maps


def _assemble(shards):
    """shards: list of 8 per-core 'out' arrays [S//TP, D] -> [B, S, D] fp32."""
    out = np.empty((B, S, D), dtype=np.float32)
    for core in range(N_CORES):
        b, g = divmod(core, TP)
        shard = np.asarray(shards[core]).astype(np.float32)
        ooff = 0
        cbase = 0
        for n in RS_CHUNKS:
            rows = n * 32
            gstart = cbase + g * rows
            out[b, gstart : gstart + rows, :] = shard[ooff : ooff + rows, :]
            ooff += rows
            cbase += n * 128
    return out


def kernel(x, Wq, Wk, Wv, Wo, token_position, trace=False, trace_cores=None):
    global LAST_RESULTS
    if trace:
        _ensure_ntff_hook()
    nc = _get_nc()
    in_maps = _shard_inputs(x, Wq, Wk, Wv, Wo, token_position)
    res = run_bass_kernel_spmd(
        nc,
        in_maps,
        core_ids=list(range(N_CORES)),
        trace=trace,
        trace_cores=trace_cores,
    )
    LAST_RESULTS = res
    return _assemble([res.results[core]["out"] for core in range(N_CORES)])
